# revision 64
# baseline (speedup 1.0000x reference)
"""Chamfer-distance loss kernel for Trainium2 (8 NeuronCores, SPMD).

Problem: loss = chamfer(coarse, gt_pts) + alpha * chamfer(fine, gt_pts)
  coarse [8,1024,3], fine [8,8192,3], gt [8,3,8192] (channel-first), alpha scalar.
  chamfer(x,y) = mean_n min_m d(n,m) + mean_m min_n d(n,m), d = squared L2.

Sharding: data-parallel over batch - one batch element per NeuronCore.

Per-core pipeline, NEGATED so every reduction is a max:
  - PE produces (2x.y - |y|^2) as a K=9 fp16 matmul, 512 cols at a time.
  - PSUM geometry: 8 chunks of [128,1024] fp32 per tile over FOUR PSUM
    slots (pool bufs=4). This is the load-bearing choice: with the old
    2x[128,2048] layout, 3 Act casts + 1 DVE drain on 2 slots always
    leave one matmul+semaphore latency (~1.2us/tile) exposed on the Act
    ring (pigeonhole over slot successions). With 4 slots and both DVE
    drains (c0, c4) on the SAME slot, every Act cast's matmul hides
    behind other casts and Act runs gap-free (measured: Act has 3 idle
    gaps total = startup + tail; engine 97% busy).
  - Drains split: chunks c0,c4 via DVE tensor_scalar (op0=add bias,
    op1=max accum_out -> fused row-max, rowWfb/rowWfb2), the other six
    via ScalarE activation casts (bias = -|x|^2 per-partition AP).
  - Row direction (min over m): per-tile 4x tensor_scalar in two column
    halves into scratch (NOT in place: scratch keeps S pristine so fold
    order is free), accum_out -> rowWfa/rowWfb1 columns. Final row =
    elementwise max of the four accumulators -> add-reduce -> Pool
    partition_all_reduce(add).
  - Col direction (min over n), fine family: alternate per tile between
      * DVE chain tiles: running elementwise max (tensor_tensor halves),
      * Pool tiles: partition_all_reduce(max) in two column halves,
        row 0 DMA-copied (SBUF->SBUF, idle DMA engines) into a stack;
        one stack AR collapses AR'd rows + the chain result.
    ~Half the fold work runs on the otherwise-idle Pool engine. Pool is
    rate-matched: >32 AR'd tiles or consecutive AR'd tiles make Pool lag
    and stall the S pool (measured, large regressions).
  - Coarse family: all-chain, no drains/ARs (Act-bound but short; drains
    make it DVE-bound = worse; coarse ARs land in Pool's backlog and
    serialize the tail - both measured). Its last tile interleaves
    row/fold per 2048-col quarter; col-collapse rails: quarters 0-2 via
    Pool AR -> outb accum, quarter 3 via PE transposes + DVE max-reduce
    (PE idle at the tail). outb cols 9..13, host sums them.
  - Startup: ~20 tiny warm-up matmuls on a memset tile finish the PE
    p-state ramp (cost model: 3us continuous-busy threshold) before the
    first real matmul; first DMAs are split so tile 0 starts ~2.6us in.

Measured (cost-model timeline, the graded metric): 482767 ns/core vs
562371 baseline (-14.2%); HW-verified rel err 1.9e-5. Tail: the last
coarse tile runs per-1024-col pieces (folds into the dead chain tile,
pieces 0-6 on the Pool AR rail, piece 7 on the PE-transpose rail with
its scalar chain inlined); rows/scr accums interleave into the piece
fronts so DVE never queues them behind the critical chain. Tile 0 is
drained too (x2f[:,0:8] is DMA'd early just for this).

Dead ends verified this session (don't retry blindly):
  - DMA accum fold: compiler rejects cce_op=max ("DMACopy does not
    support max with Copy mode"); only gpsimd SWDGE can accum at all.
  - Matmul fp16->PSUM (would give 2x DVE drains): TRN3+ only, asserted.
  - dma_start from/to PSUM: not supported (SBUF/DRAM only).
  - Activation accum_out: accumulates, but no engine-time saving vs the
    op cost; bias APs are free (scalar operands exempt in cost model).
  - tensor_tensor_reduce: hard-crashes the exec unit (prev session).
  - GPSIMD tensor_tensor/tensor_scalar/tensor_reduce: fail to compile.
  - Busy-balance moves (c2-drains, denser ARs, coarse drains/ARs) that
    look good on paper lose to scheduling dynamics; the per-tile slot
    rings and Pool lag dominate. Measure via analyze.py, not LP.

Host does only O(N) prep (transpose/cast/aug-row construction, negation)
and the final scalar arithmetic (negating the totals back).
"""

import sys

sys.path.insert(0, "/opt/trn_rl_repo")

import numpy as np

B = 8
NF = 8192  # fine points
NC_ = 1024  # coarse points
M = 8192  # gt points

NTF = NF // 128  # 64 fine tiles
NTC = NC_ // 128  # 8 coarse tiles

# ---- knobs -----------------------------------------------------------------
# Fine tiles whose g0 chunk is DVE-drained (tensor_scalar w/ fused row accum).
DVE_DRAIN_FINE = set(range(0, NTF))
# Fine tiles whose col-fold goes to Pool (partition_all_reduce) instead of the
# DVE chain. Must NOT include the last fine tile (chain must end the family).
AR_FINE = set(i for i in range(2, NTF - 2) if i % 2 == 0)
# Coarse tiles whose g0 chunk is DVE-drained (tail tile NTC-1 never).
DVE_DRAIN_COARSE = set()
# Coarse tiles AR'd into stackC (fold work to Pool's late idle window).
# Never the chain-start tile (NTC-2) or the tail tile.
AR_COARSE = set()
# Split factors for DVE fold / row ops (more pieces = finer interleaving
# with the latency-critical PSUM drains, slightly more per-op overhead).
FOLD_SPLIT = 2
ROW_SPLIT = 2  # 1 or 2 only (accumulator columns alternate A/B)
DRAIN_HALVES = False
INLINE_DRAIN = False
# Tail col-collapse rails per quarter: 'P' = Pool partition_all_reduce,
# 'T' = PE transposes + DVE max-reduce.
TAIL_RAILS = 'PPPT'
T_PIECES = {7}
# Split the g1 cast into [512,1536]: the 512 piece needs only the first
# matmul of its chunk, shortening the exposed-matmul gap after g3(i-1).
SPLIT_G1 = False
DIAG_NO_ROWS = False
DIAG_NO_FOLDS = False
DIAG_NO_DRAIN_ACCUM = False
# Fine tiles where the running DVE chain is broken: the old chain is AR'd
# into its own stack row and a new chain starts (spreads Pool work).
FINE_CHAIN_BREAKS = set()
# Fine tiles that additionally DVE-drain chunk c2 (slot 2).
DVE_DRAIN_C2 = set()
# Fine tiles that drain c1 as well (third DVE drain, rows shrink).
FINE_EXTRA = set()
N_WARM = 20
S_BUFS = 6
AR_BUFS = 2

# --- module-level program cache -------------------------------------------
_PROGRAM = None
PROFILE = False
LAST_RESULTS = None


def _build_program():
    from concourse import bacc, bass, tile, bass_isa
    import concourse.mybir as mybir

    f16, f32 = mybir.dt.float16, mybir.dt.float32
    AL = mybir.AluOpType
    ACTF = mybir.ActivationFunctionType
    RED = bass_isa.ReduceOp

    nc = bacc.Bacc("TRN2", target_bir_lowering=False, debug=False, num_devices=B)

    yaug_d = nc.dram_tensor("yaug", [9, M], f16, kind="ExternalInput")
    xaug_f = nc.dram_tensor("xaug_f", [9, NF], f16, kind="ExternalInput")
    xaug_c = nc.dram_tensor("xaug_c", [9, NC_], f16, kind="ExternalInput")
    x2f_d = nc.dram_tensor("x2nf", [128, NTF], f32, kind="ExternalInput")
    x2c_d = nc.dram_tensor("x2nc", [128, NTC], f32, kind="ExternalInput")
    iden_d = nc.dram_tensor("iden", [128, 128], f16, kind="ExternalInput")
    out_d = nc.dram_tensor("out", [1, 16], f32, kind="ExternalOutput")

    n_stackF = len(AR_FINE) + 1 + len(FINE_CHAIN_BREAKS)
    n_stackC = len(AR_COARSE) + 1

    with tile.TileContext(nc) as tc:
        with (
            tc.tile_pool(name="const", bufs=1) as cpool,
            tc.tile_pool(name="s", bufs=S_BUFS) as spool,
            tc.tile_pool(name="scr", bufs=1) as scrpool,
            tc.tile_pool(name="arout", bufs=AR_BUFS) as apool,
            tc.tile_pool(name="fin", bufs=1) as fpool,
            tc.tile_pool(name="ps", bufs=4, space=bass.MemorySpace.PSUM) as pspool,
        ):
            Y = cpool.tile([9, M], f16)
            nc.sync.dma_start(Y[:, 0:2048], yaug_d.ap()[:, 0:2048])
            Xf = cpool.tile([9, NF], f16)
            nc.sync.dma_start(Xf[:, 0:128], xaug_f.ap()[:, 0:128])
            x2f = cpool.tile([128, NTF], f32)
            nc.sync.dma_start(x2f[:, 0:8], x2f_d.ap()[:, 0:8])
            nc.sync.dma_start(Y[:, 2048:4096], yaug_d.ap()[:, 2048:4096])
            nc.sync.dma_start(Xf[:, 128:1024], xaug_f.ap()[:, 128:1024])
            nc.sync.dma_start(x2f[:, 8:NTF], x2f_d.ap()[:, 8:NTF])
            nc.sync.dma_start(Y[:, 4096:M], yaug_d.ap()[:, 4096:M])
            nc.sync.dma_start(Xf[:, 1024:NF], xaug_f.ap()[:, 1024:NF])
            Xc = cpool.tile([9, NC_], f16)
            nc.sync.dma_start(Xc[:], xaug_c.ap())
            x2c = cpool.tile([128, NTC], f32)
            nc.sync.dma_start(x2c[:], x2c_d.ap())
            iden = cpool.tile([128, 128], f16)
            nc.sync.dma_start(iden[:], iden_d.ap())

            outb = cpool.tile([1, 16], f32)
            nc.gpsimd.memset(outb[:], 0.0)

            # PE p-state warm-up: ~3.5us of tiny back-to-back matmuls on a
            # memset tile finish the clock ramp before the first real matmul.
            pewarm = cpool.tile([1, 128], f16)
            nc.gpsimd.memset(pewarm[:], 0.0)
            for _ in range(N_WARM // 2):
                pswarm = pspool.tile([1, 128], f32, tag="ps")
                for _ in range(2):
                    nc.tensor.matmul(
                        pswarm[:], lhsT=pewarm[0:1, 0:1], rhs=pewarm[:],
                        start=True, stop=True,
                    )

            # Row accumulators: one column per tile; halves A/B from the
            # split row TS, rowWb from fused drain accums.
            rowWfa = cpool.tile([128, NTF], f32)
            rowWfb1 = cpool.tile([128, NTF], f32)
            rowWfb = cpool.tile([128, NTF], f32)
            nc.gpsimd.memset(rowWfb[:], -60000.0)
            rowWfb2 = cpool.tile([128, NTF], f32)
            nc.gpsimd.memset(rowWfb2[:], -60000.0)
            rowWfb3 = cpool.tile([128, NTF], f32)
            nc.gpsimd.memset(rowWfb3[:], -60000.0)
            rowWca = cpool.tile([128, NTC + 1], f32)
            rowWcb1 = cpool.tile([128, NTC + 1], f32)
            if DIAG_NO_ROWS:
                for t_ in (rowWfa, rowWfb1, rowWca, rowWcb1):
                    nc.gpsimd.memset(t_[:], -60000.0)
            rowWcb = cpool.tile([128, NTC], f32)
            nc.gpsimd.memset(rowWcb[:], -60000.0)
            rowWcb2 = cpool.tile([128, NTC], f32)
            nc.gpsimd.memset(rowWcb2[:], -60000.0)

            # One stack tile for both families: cost is free-size driven
            # (16KB regardless of partition count), so disjoint partition
            # ranges are free. Fine rows start at 0, coarse rows at 48.
            stacks = cpool.tile([48 + n_stackC, M], f16)
            stackF = stacks
            stackC = stacks

            # Force the Identity act-table load before the pipeline starts.
            warm = cpool.tile([1, 1], f16)
            nc.scalar.activation(
                warm[:], outb[0:1, 0:1], ACTF.Identity, bias=0.0, scale=1.0
            )

            H = M // 2

            def emit_tile_front(Xa, x2, i, S, dve_drain, rowWb, rowWb2=None,
                                drain_c2=False, rowWb3=None,
                                drain_chunks=(0, 4)):
                """Matmuls + casts for tile i in 8 chunks of 1024 columns over
                4 PSUM slots. Chunks c0 and c4 (which share a PSUM slot) are
                DVE-drained when dve_drain: Act then never waits on a
                drain-freed slot, removing the exposed-matmul ring latency
                that a 2-slot layout forces. Returns deferred drain closures.
                """
                deferred = []
                for c in range(8):
                    ps = pspool.tile([128, 1024], f32, tag="ps")
                    for j in range(2):
                        mlo = c * 1024 + j * 512
                        nc.tensor.matmul(
                            ps[:, j * 512 : (j + 1) * 512],
                            lhsT=Xa[:, i * 128 : (i + 1) * 128],
                            rhs=Y[:, mlo : mlo + 512],
                            start=True,
                            stop=True,
                        )
                    if (dve_drain and c in drain_chunks) or (drain_c2 and c == 2):
                        def mk(ps=ps, c=c):
                            w = {0: rowWb, 1: rowWb3, 2: rowWb3, 4: rowWb2}[c]
                            nc.vector.tensor_scalar(
                                out=S[:, c * 1024 : (c + 1) * 1024],
                                in0=ps[:],
                                scalar1=x2[:, i : i + 1],
                                scalar2=None,
                                op0=AL.add,
                                op1=AL.max,
                                accum_out=w[:, i : i + 1],
                            )
                        deferred.append(mk)
                    else:
                        nc.scalar.activation(
                            S[:, c * 1024 : (c + 1) * 1024],
                            ps[:],
                            ACTF.Identity,
                            bias=x2[:, i : i + 1],
                            scale=1.0,
                        )
                return deferred

            def emit_row(S, rowWa, rowWb1, i, drained):
                # Row-max in split column pieces (finer DVE ops keep the
                # latency-critical PSUM drains from queueing behind them).
                # Pieces alternate accumulators A/B (combined at family end).
                # Output goes to a scratch tile: writing S in place would
                # serialize the fold TT / Pool AR (which read S) behind the
                # row op. Scratch WAR reuse only orders DVE-internal ops.
                lo = 2048 if drained else 0
                scr = scrpool.tile([128, H], f16, tag="scr")
                # chunk-aligned cuts: a piece must not straddle an extra
                # cast chunk or it waits on casts it doesn't need.
                cuts = [lo, H, M] if ROW_SPLIT == 2 else [lo, M]
                for k in range(len(cuts) - 1):
                    w = rowWa if k % 2 == 0 else rowWb1
                    nc.vector.tensor_scalar(
                        out=scr[:, 0 : cuts[k + 1] - cuts[k]],
                        in0=S[:, cuts[k] : cuts[k + 1]],
                        scalar1=-60000.0,
                        scalar2=None,
                        op0=AL.max,
                        op1=AL.max,
                        accum_out=w[:, i : i + 1],
                    )

            def emit_ar(S, stack, row):
                """Pool partition_all_reduce in two halves + DMA stack copies."""
                for h in range(2):
                    ar = apool.tile([128, H], f16, tag="ar")
                    nc.gpsimd.partition_all_reduce(
                        ar[:], S[:, h * H : (h + 1) * H],
                        channels=128, reduce_op=RED.max,
                    )
                    nc.sync.dma_start(
                        stack[row : row + 1, h * H : (h + 1) * H], ar[0:1, :]
                    )

            # ---------------- fine family ----------------
            chain = None  # AP of the running chain S tile
            S_of = {}
            pending_drain = None
            ar_row = [0]

            def emit_back(i, fam_S, rowWa, rowWb1, drained_set, ar_set, stack):
                """Deferred work for tile i (emitted while tile i+1's matmuls
                run). Row pieces and fold halves interleave [rowA, foldA,
                rowB, foldB] so each fold half enters the DVE queue as early
                as its columns allow (shortens the chain-latency tail)."""
                nonlocal chain
                S = fam_S[i]
                drained = i in drained_set
                is_ar = ar_set is not None and i in ar_set
                # One row piece per column half. For c2-drained tiles the
                # half-A piece re-covers the drained [2048:3072] columns:
                # harmless (max is idempotent) and keeps two accumulators.
                if drained and drained_set is DVE_DRAIN_COARSE:
                    pieces = ((1024, H), (H, M))
                elif drained and i in FINE_EXTRA:
                    pieces = ((2048, H), (5120, M))
                elif drained:
                    pieces = ((1024, H), (5120, M))
                else:
                    pieces = ((0, H), (H, M))
                scr = scrpool.tile([128, H], f16, tag="scr")
                for k, (a, b) in enumerate(pieces):
                    w = rowWa if k % 2 == 0 else rowWb1
                    if not DIAG_NO_ROWS:
                        nc.vector.tensor_scalar(
                            out=scr[:, 0 : b - a],
                            in0=S[:, a:b],
                            scalar1=-60000.0,
                            scalar2=None,
                            op0=AL.max,
                            op1=AL.max,
                            accum_out=w[:, i : i + 1],
                        )
                    if is_ar:
                        ar = apool.tile([128, H], f16, tag="ar", bufs=AR_BUFS)
                        nc.gpsimd.partition_all_reduce(
                            ar[:], S[:, k * H : (k + 1) * H],
                            channels=128, reduce_op=RED.max,
                        )
                        nc.sync.dma_start(
                            stack[ar_row[0] : ar_row[0] + 1,
                                  k * H : (k + 1) * H],
                            ar[0:1, :],
                        )
                    elif chain is not None and not DIAG_NO_FOLDS:
                        nc.vector.tensor_tensor(
                            out=S[:, k * H : (k + 1) * H],
                            in0=S[:, k * H : (k + 1) * H],
                            in1=chain[:, k * H : (k + 1) * H],
                            op=AL.max,
                        )
                if is_ar:
                    ar_row[0] += 1
                    del fam_S[i]
                else:
                    chain = S

            for i in range(NTF):
                if i in FINE_CHAIN_BREAKS and chain is not None:
                    emit_ar(chain, stackF, ar_row[0])
                    ar_row[0] += 1
                    chain = None
                S = spool.tile([128, M], f16, tag="S")
                S_of[i] = S
                dve_d = i in DVE_DRAIN_FINE
                dc = (0, 1, 4) if i in FINE_EXTRA else (0, 4)
                deferred = emit_tile_front(Xf, x2f, i, S, dve_d, rowWfb,
                                           rowWfb2,
                                           drain_c2=(i in DVE_DRAIN_C2),
                                           rowWb3=rowWfb3,
                                           drain_chunks=dc)
                if pending_drain:
                    for fn in pending_drain:
                        fn()
                pending_drain = deferred
                if i >= 1:
                    emit_back(i - 1, S_of, rowWfa, rowWfb1, DVE_DRAIN_FINE,
                              AR_FINE, stackF)
            if pending_drain:
                for fn in pending_drain:
                    fn()
                pending_drain = None
            emit_back(NTF - 1, S_of, rowWfa, rowWfb1, DVE_DRAIN_FINE,
                      AR_FINE, stackF)
            # fine chain final: AR into the stack's last row (two halves)
            emit_ar(chain, stackF, n_stackF - 1)
            chain = None

            # fine row total: max of the three accumulators -> free add-reduce
            # -> partition AR add.
            nc.vector.tensor_tensor(
                out=rowWfa[:], in0=rowWfa[:], in1=rowWfb1[:], op=AL.max
            )
            nc.vector.tensor_tensor(
                out=rowWfa[:], in0=rowWfa[:], in1=rowWfb[:], op=AL.max
            )
            nc.vector.tensor_tensor(
                out=rowWfa[:], in0=rowWfa[:], in1=rowWfb2[:], op=AL.max
            )
            nc.vector.tensor_tensor(
                out=rowWfa[:], in0=rowWfa[:], in1=rowWfb3[:], op=AL.max
            )
            rsf = fpool.tile([128, 1], f32, tag="rsf")
            nc.vector.tensor_reduce(
                out=rsf[:], in_=rowWfa[:], axis=mybir.AxisListType.X, op=AL.add
            )
            rsumf = fpool.tile([128, 1], f32, tag="rsumf")
            with tc.high_priority(offset=-200):
                # Tiny op: jump the Pool queue ahead of queued tile-ARs so
                # its DVE consumer (outb copy) doesn't wait the backlog.
                nc.gpsimd.partition_all_reduce(
                    rsumf[:], rsf[:], channels=128, reduce_op=RED.add
                )

            # ---------------- coarse family ----------------
            Sc_of = {}
            ar_row = [48]

            stmax_box = [None]

            def fine_finals_pool():
                # fine col: collapse the stack (overlaps the coarse family).
                stmax = fpool.tile([n_stackF, M], f16, tag="stmax")
                stmax_box[0] = stmax
                for h in range(2):
                    nc.gpsimd.partition_all_reduce(
                        stmax[:, h * H : (h + 1) * H],
                        stackF[0:n_stackF, h * H : (h + 1) * H],
                        channels=n_stackF, reduce_op=RED.max,
                    )

            def fine_finals_dve():
                # Emitted a few tiles later: the Pool backlog means stmax /
                # rsumf complete late, and DVE must not head-block on them.
                stmax = stmax_box[0]
                nc.vector.tensor_copy(outb[0:1, 0:1], rsumf[0:1, 0:1])
                nc.vector.tensor_scalar(
                    out=stmax[0:1, :],
                    in0=stmax[0:1, :],
                    scalar1=-60000.0,
                    scalar2=None,
                    op0=AL.max,
                    op1=AL.add,
                    accum_out=outb[0:1, 1:2],
                )

            for i in range(NTC - 1):
                S = spool.tile([128, M], f16, tag="S")
                Sc_of[i] = S
                deferred = emit_tile_front(Xc, x2c, i, S,
                                           i in DVE_DRAIN_COARSE, rowWcb,
                                           rowWcb2, drain_chunks=(0,))
                if pending_drain:
                    for fn in pending_drain:
                        fn()
                pending_drain = deferred
                if i >= 1:
                    emit_back(i - 1, Sc_of, rowWca, rowWcb1, DVE_DRAIN_COARSE,
                              AR_COARSE, stackC)
                else:
                    # first coarse tile's matmuls are in flight; fine finals
                    # Pool work queues behind the fine AR backlog.
                    fine_finals_pool()
                if i == 4:
                    # DVE consumers emitted late so DVE doesn't head-block
                    # waiting for the Pool backlog to drain.
                    fine_finals_dve()
            if pending_drain:
                for fn in pending_drain:
                    fn()
                pending_drain = None
            # Last chained tile: folds first (its rows are off the critical
            # path and run during the tail), so the tail's prev is ready at
            # the earliest possible moment. If every earlier coarse tile was
            # AR'd there is no chain yet: normal emit_back (its rows execute
            # on DVE before the tail folds overwrite S6 -- in-order queue).
            S6 = Sc_of[NTC - 2]
            if chain is None:
                emit_back(NTC - 2, Sc_of, rowWca, rowWcb1, DVE_DRAIN_COARSE,
                          None, None)
                late_rows = []
            else:
                for k in range(2):
                    nc.vector.tensor_tensor(
                        out=chain[:, k * H : (k + 1) * H],
                        in0=chain[:, k * H : (k + 1) * H],
                        in1=S6[:, k * H : (k + 1) * H],
                        op=AL.max,
                    )
                late_rows = [(S6, NTC - 2)]

            # Last coarse tile: per-quarter pipeline. Front work (mms +
            # cast) for quarter g is emitted before the tail work of quarter
            # g-1 so PE/Act never queue behind DVE/Pool tail ops. Each
            # quarter: row-max, fold into chain, AR the chain columns, DMA
            # into stackC's last row, collapse stackC columns, accumulate the
            # column total into outb (cols 9..12, host sums them).
            i = NTC - 1
            S = spool.tile([128, M], f16, tag="S")
            prev = chain
            stmaxC = None
            if AR_COARSE:
                stmaxC = fpool.tile([n_stackC, M], f16, tag="stmax")
            nT_rail = 0 if AR_COARSE else TAIL_RAILS.count("T")
            cmb = None
            if nT_rail:
                cmb = fpool.tile([128, 16 * nT_rail], f16, tag="cmb")
            t_blk = [0]

            def tail_work(g):
                lo, hi = g * 2048, (g + 1) * 2048
                # row-max: quarters q0/q2 -> rowWca cols {i, NTC}, q1/q3 ->
                # rowWcb1 cols {i, NTC}; scratch col combined after q3.
                w = rowWca if g % 2 == 0 else rowWcb1
                c = i if g < 2 else NTC
                scr = scrpool.tile([128, H], f16, tag="scr")
                nc.vector.tensor_scalar(
                    out=scr[:, 0:2048],
                    in0=S[:, lo:hi],
                    scalar1=-60000.0,
                    scalar2=None,
                    op0=AL.max,
                    op1=AL.max,
                    accum_out=w[:, c : c + 1],
                )
                if g == 3:
                    for w2 in (rowWca, rowWcb1):
                        nc.vector.tensor_tensor(
                            out=w2[:, i : i + 1],
                            in0=w2[:, i : i + 1],
                            in1=w2[:, NTC : NTC + 1],
                            op=AL.max,
                        )
                nc.vector.tensor_tensor(
                    out=S[:, lo:hi], in0=S[:, lo:hi],
                    in1=prev[:, lo:hi], op=AL.max,
                )
                if not AR_COARSE and TAIL_RAILS[g] == 'T':
                    # PE rail: transposes + DVE max-reduce (PE and PSUM are
                    # idle at the tail; Pool is draining its fine backlog).
                    pst = pspool.tile([128, 16, 128], f16, tag="ps")
                    for q in range(16):
                        nc.tensor.transpose(
                            pst[:, q, :],
                            S[:, lo + q * 128 : lo + (q + 1) * 128],
                            iden[:],
                        )
                    blk = t_blk[0]
                    t_blk[0] += 1
                    nc.vector.tensor_reduce(
                        out=cmb[:, blk * 16 : (blk + 1) * 16],
                        in_=pst[:],
                        axis=mybir.AxisListType.X,
                        op=AL.max,
                    )
                    return
                arq = apool.tile([128, 2048], f16, tag="ar")
                nc.gpsimd.partition_all_reduce(
                    arq[:], S[:, lo:hi], channels=128, reduce_op=RED.max
                )
                if AR_COARSE:
                    nc.sync.dma_start(
                        stackC[48 + n_stackC - 1 : 48 + n_stackC, lo:hi],
                        arq[0:1, :],
                    )
                    nc.gpsimd.partition_all_reduce(
                        stmaxC[:, lo:hi],
                        stackC[48 : 48 + n_stackC, lo:hi],
                        channels=n_stackC, reduce_op=RED.max,
                    )
                    nc.vector.tensor_scalar(
                        out=stmaxC[0:1, lo:hi],
                        in0=stmaxC[0:1, lo:hi],
                        scalar1=-60000.0,
                        scalar2=None,
                        op0=AL.max,
                        op1=AL.add,
                        accum_out=outb[0:1, 9 + g : 10 + g],
                    )
                else:
                    nc.vector.tensor_scalar(
                        out=arq[0:1, :],
                        in0=arq[0:1, :],
                        scalar1=-60000.0,
                        scalar2=None,
                        op0=AL.max,
                        op1=AL.add,
                        accum_out=outb[0:1, 9 + g : 10 + g],
                    )

            for g in range(4):
                ps = pspool.tile([128, 2048], f32, tag="ps")
                for j in range(4):
                    mlo = g * 2048 + j * 512
                    nc.tensor.matmul(
                        ps[:, j * 512 : (j + 1) * 512],
                        lhsT=Xc[:, i * 128 : (i + 1) * 128],
                        rhs=Y[:, mlo : mlo + 512],
                        start=True,
                        stop=True,
                    )
                lo, hi = g * 2048, (g + 1) * 2048
                nc.scalar.activation(
                    S[:, lo:hi], ps[:], ACTF.Identity,
                    bias=x2c[:, i : i + 1], scale=1.0,
                )
                if g >= 1:
                    tail_work(g - 1)
            tail_work(3)
            if nT_rail:
                # PE-rail total: sum of per-m col-maxes -> free add-reduce ->
                # partition AR add.
                csum = fpool.tile([128, 1], f32, tag="csum")
                nc.vector.tensor_reduce(
                    out=csum[:], in_=cmb[:], axis=mybir.AxisListType.X,
                    op=AL.add,
                )
                csumT = fpool.tile([128, 1], f32, tag="csumT")
                nc.gpsimd.partition_all_reduce(
                    csumT[:], csum[:], channels=128, reduce_op=RED.add
                )
                nc.vector.tensor_copy(outb[0:1, 11:12], csumT[0:1, 0:1])
            chain = None

            # coarse row total: combine halves + drain accums, then reduce.
            nc.vector.tensor_tensor(
                out=rowWca[:, 0:NTC], in0=rowWca[:, 0:NTC],
                in1=rowWcb1[:, 0:NTC], op=AL.max,
            )
            nc.vector.tensor_tensor(
                out=rowWca[:, 0:NTC], in0=rowWca[:, 0:NTC],
                in1=rowWcb[:], op=AL.max,
            )
            nc.vector.tensor_tensor(
                out=rowWca[:, 0:NTC], in0=rowWca[:, 0:NTC],
                in1=rowWcb2[:], op=AL.max,
            )
            rsc = fpool.tile([128, 1], f32, tag="rsc")
            nc.vector.tensor_reduce(
                out=rsc[:], in_=rowWca[:, 0:NTC], axis=mybir.AxisListType.X,
                op=AL.add,
            )
            rsumc = fpool.tile([128, 1], f32, tag="rsumc")
            nc.gpsimd.partition_all_reduce(
                rsumc[:], rsc[:], channels=128, reduce_op=RED.add
            )
            nc.vector.tensor_copy(outb[0:1, 8:9], rsumc[0:1, 0:1])

            nc.sync.dma_start(out_d.ap(), outb[:])

    nc.compile()
    return nc


def _get_program():
    global _PROGRAM
    if _PROGRAM is None:
        _PROGRAM = _build_program()
    return _PROGRAM


def _prep_core_inputs(fine_b, coarse_b, gt_b):
    f16 = np.float16
    xf = np.ones((9, NF), f16)
    xf[0:3] = fine_b.astype(f16).T
    xc = np.ones((9, NC_), f16)
    xc[0:3] = coarse_b.astype(f16).T
    g16 = gt_b.astype(f16)  # [3, M]
    yaug = np.empty((9, M), f16)
    yaug[0:3] = (2.0 * g16.astype(np.float32)).astype(f16)
    sq = -(g16.astype(np.float32) ** 2)
    hi = sq.astype(f16)
    yaug[3:6] = hi
    yaug[6:9] = (sq - hi.astype(np.float32)).astype(f16)
    x2f = -(fine_b.astype(f16).astype(np.float32) ** 2).sum(1).reshape(-1, 128).T
    x2c = -(coarse_b.astype(f16).astype(np.float32) ** 2).sum(1).reshape(-1, 128).T
    return {
        "xaug_f": xf,
        "xaug_c": xc,
        "yaug": yaug,
        "x2nf": np.ascontiguousarray(x2f, np.float32),
        "x2nc": np.ascontiguousarray(x2c, np.float32),
        "iden": np.eye(128, dtype=f16),
    }


def kernel(coarse, fine, gt, alpha):
    global LAST_RESULTS
    from concourse import bass_utils

    coarse = np.asarray(coarse, np.float32)
    fine = np.asarray(fine, np.float32)
    gt = np.asarray(gt, np.float32)
    alpha = np.float32(np.asarray(alpha))

    nc = _get_program()
    in_maps = [_prep_core_inputs(fine[b], coarse[b], gt[b]) for b in range(B)]
    try:
        res = bass_utils.run_bass_kernel_spmd(
            nc, in_maps, core_ids=list(range(B)), trace=PROFILE
        )
    except Exception:
        # One retry: a transiently wedged NeuronCore recovers on the next
        # attempt - observed once on this runtime.
        res = bass_utils.run_bass_kernel_spmd(
            nc, in_maps, core_ids=list(range(B)), trace=PROFILE
        )
    LAST_RESULTS = res
    per = np.stack([r["out"][0] for r in res.results]).astype(np.float64)  # [B,16]
    rowf = -per[:, 0]
    colf = -per[:, 1]
    rowc = -per[:, 8]
    colc = -(per[:, 9] + per[:, 10] + per[:, 11] + per[:, 12])
    lf = np.float32((rowf / NF + colf / M).mean())
    lc = np.float32((rowc / NC_ + colc / M).mean())
    loss = np.float32(lc + np.float32(alpha) * lf)
    return (loss, lc, lf)


if __name__ == "__main__":
    rng = np.random.default_rng(0)
    out = kernel(
        coarse=rng.standard_normal((B, NC_, 3)).astype(np.float32),
        fine=rng.standard_normal((B, NF, 3)).astype(np.float32),
        gt=rng.standard_normal((B, 3, M)).astype(np.float32),
        alpha=np.float32(1.0),
    )
    print(out)
